# revision 34
# baseline (speedup 1.0000x reference)
"""Trainium2 Bass kernel for the contrastive loss problem.

Math reformulation of the reference (no [N, 2N-1] scatter needed):
  lse_i = log( exp(pos_val_i) + sum_{j in neg} exp(S_ij) + (2N-2-num_neg_i) )
  loss  = mean_i (lse_i - pos_val_i)
with S = (cos + 1) * 0.25, cos from row-normalized embeddings.

Sharding uses the Gram matrix's symmetry: core c computes only the
[512, 512*5] strip of exp(S) pairing its rows with block-columns
{c, c+1, .., c+4} (mod 8). Columns are pre-rotated on the host so the
program is identical on every core (SPMD). Row sums cover the strip;
one-hot-weight matmuls produce per-column sums for the foreign blocks
(distance 1..3), which the host adds to those rows' totals. Distance-4
blocks are computed by both endpoint cores (row sums only).

Main matmul: fp8 e4m3 DoubleRow (K=256/op) on x16-prescaled unit rows.
Input delivery (et, 2.6MB) is HBM-bound (~10us): the first row-chunk
pair is computed contraction-outer over 7 block-chains (PSUM-capped)
so each et chunk pair is consumed as it lands, with filler matmuls
holding the PE p-state between chunk arrivals. Blocks are processed in
order [1,2,3,0,4] so the column-sum operands are ready early; exp and
mask run as paired 1024-wide ops; exp output is stored fp8 stacked
across m-chunk pairs so the column-sum matmuls are DoubleRow with
one-hot weights into a single PSUM bank (partitions 0:6).

Host: norms, fp8/bf16 casts, rotation, first-positive gather (label
metadata), final assembly of ~4096 scalars.
"""

import sys

sys.path.insert(0, "/opt/trn_rl_repo")

from contextlib import ExitStack

import ml_dtypes
import numpy as np

import concourse.bacc as bacc
import concourse.tile as tile
from concourse import mybir
from concourse.bass_utils import run_bass_kernel_spmd

N, D = 4096, 1024
NCORES = 8
R = N // NCORES            # 512 rows per core
P = 128                    # partitions
MI = R // P                # 4 row chunks per core
KC = D // P                # 8 contraction chunks
JW = 512                   # j tile width (one PSUM bank)
NB = 5                     # block-columns per core (self + 4 right neighbors)
JCOLS = NB * JW            # 2560
EPS = 1e-8
BF16 = ml_dtypes.bfloat16
FP8 = ml_dtypes.float8_e4m3
SCALE = 16.0
SLOT2BLK = [1, 2, 3, 0, 4]  # processing order: colsum blocks first

_CACHE = {}


def _build_program():
    nc = bacc.Bacc("TRN2", target_bir_lowering=False, debug=False)
    f32, bf16, fp8 = mybir.dt.float32, mybir.dt.bfloat16, mybir.dt.float8e4
    AF = mybir.ActivationFunctionType
    OP = mybir.AluOpType
    DR = mybir.MatmulPerfMode.DoubleRow

    et_d = nc.dram_tensor("et", [KC, P, JCOLS], fp8, kind="ExternalInput")
    yt_d = nc.dram_tensor("yt", [P, JCOLS], bf16, kind="ExternalInput")
    yb_d = nc.dram_tensor("yb", [P, MI], f32, kind="ExternalInput")
    enef_d = nc.dram_tensor("enef", [MI, P, 2 * D], bf16, kind="ExternalInput")
    ef8_d = nc.dram_tensor("ef8", [KC, P, P], fp8, kind="ExternalInput")
    eye_d = nc.dram_tensor("eye", [P, P], fp8, kind="ExternalInput")
    ro_d = nc.dram_tensor("rowout", [P, 2 * MI], f32, kind="ExternalOutput")
    cs_d = nc.dram_tensor("csout", [6, JW], f32, kind="ExternalOutput")

    with tile.TileContext(nc) as tc, ExitStack() as ctx:
        const = ctx.enter_context(tc.tile_pool(name="const", bufs=1))
        psum = ctx.enter_context(tc.tile_pool(name="psum", bufs=3, space="PSUM"))
        psum1 = ctx.enter_context(tc.tile_pool(name="psum1", bufs=1,
                                               space="PSUM"))
        cspsum = ctx.enter_context(tc.tile_pool(name="cspsum", bufs=1,
                                                space="PSUM"))
        esp = ctx.enter_context(tc.tile_pool(name="esp", bufs=2))
        work = ctx.enter_context(tc.tile_pool(name="work", bufs=2))
        acc = ctx.enter_context(tc.tile_pool(name="acc", bufs=2))

        et = const.tile([P, KC, JCOLS], fp8, tag="et")
        yt = const.tile([P, JCOLS], bf16, tag="yt")
        yb = const.tile([P, MI], f32, tag="yb")
        enef = const.tile([P, MI, 2, D], bf16, tag="enef")
        b025 = const.tile([P, 1], f32, tag="b025")
        nc.vector.memset(b025, 0.25)
        ones = const.tile([P, 1], bf16, tag="ones")
        nc.gpsimd.memset(ones, 1.0)
        wsrc = const.tile([P, 4 * P], bf16, tag="wsrc")
        nc.gpsimd.memset(wsrc, 1.0)
        # one-hot DoubleRow weights: wcs[d][:, :, 0:6] has a 1 in column d.
        # 16-wide padding keeps the k-pair stride a multiple of 16 (DR ISA).
        wcs = [const.tile([P, 2, 16], fp8, tag=f"wc{d}", name=f"wc{d}")
               for d in range(6)]
        for d in range(6):
            nc.gpsimd.memset(wcs[d], 0.0)
            nc.gpsimd.memset(wcs[d][:, :, d:d + 1], 1.0)
        rowout = const.tile([P, 2 * MI], f32, tag="rowout")
        nsout = rowout[:, 0:MI]
        pdout = rowout[:, MI:2 * MI]
        cs = cspsum.tile([P, JW], f32, tag="cs")

        def warmup(n):
            # keep the PE clock hot while waiting on input DMAs: tiny
            # matmuls into a partition strip the column sums never touch
            for _ in range(n):
                nc.tensor.matmul(
                    cs[96:97, 0:4 * P], ones, wsrc, start=True, stop=True,
                    tile_position=(0, 96), skip_group_check=True,
                )

        # Input DMAs on the sync HW DGE queue (HBM bandwidth is the shared
        # cap; a second queue doesn't speed aggregate delivery). et chunks
        # first, in contraction order; yt split so the first masks can
        # start right at delivery end.
        for k in range(KC):
            nc.sync.dma_start(out=et[:, k, :], in_=et_d[k])
        nc.sync.dma_start(out=yt[:, 0:3 * JW], in_=yt_d[:, 0:3 * JW])
        nc.sync.dma_start(out=yb, in_=yb_d[:])
        nc.sync.dma_start(out=yt[:, 3 * JW:JCOLS], in_=yt_d[:, 3 * JW:JCOLS])
        for m in range(MI):
            nc.sync.dma_start(out=enef[:, m, :, :], in_=enef_d[m])
        ef8 = const.tile([P, KC, P], fp8, tag="ef8")
        eye = const.tile([P, P], fp8, tag="eye")
        nc.sync.dma_start(out=ef8, in_=ef8_d[:, :, :])
        nc.sync.dma_start(out=eye, in_=eye_d[:, :])
        # load the Exp table while waiting on DMAs
        warm = const.tile([P, 1], f32, tag="warm")
        nc.scalar.activation(warm, b025, AF.Exp, bias=b025, scale=1.0)

        def mmul(pt, m, s, k2, start, stop):
            b = SLOT2BLK[s]
            nc.tensor.matmul(
                pt,
                et[:, 2 * k2:2 * k2 + 2, m * P:(m + 1) * P],
                et[:, 2 * k2:2 * k2 + 2, b * JW:(b + 1) * JW],
                start=start, stop=stop, perf_mode=DR,
            )

        def expmask(esv, mmv, pt, m, mh, s, w, t1, t2, slot):
            # expS = exp(cos*0.25 + 0.25); t1[:, slot] = row-sum
            nc.scalar.activation(
                esv[:, mh, s:s + w, :], pt, AF.Exp, bias=b025,
                scale=0.25 / (SCALE * SCALE),
                accum_out=t1[:, slot:slot + 1],
            )
            # t2[:, slot] = row-sum((y == y_row) * expS)
            nc.vector.scalar_tensor_tensor(
                mmv[:, mh, s:s + w, :], yt[:, s * JW:(s + w) * JW],
                yb[:, m:m + 1], esv[:, mh, s:s + w, :],
                op0=OP.is_equal, op1=OP.mult,
                accum_out=t2[:, slot:slot + 1],
            )

        def emit_pdo(mq):
            # pdout[:, mq] = row-wise <e_i, e_firstpos(i)>; m=3's dot is
            # slotted before the trailing mask so it never gates the
            # column-sum chain nor the rowout DMA
            pdo = work.tile([P, D], bf16, tag="pdo", name="pdo")
            nc.vector.scalar_tensor_tensor(
                pdo, enef[:, mq, 0, :], 1.0, enef[:, mq, 1, :],
                op0=OP.mult, op1=OP.mult,
                accum_out=pdout[:, mq:mq + 1],
            )

        for mp in range(2):
            es = esp.tile([P, 2, NB, JW], fp8, tag="es", name="es")
            mm = esp.tile([P, 2, NB, JW], fp8, tag="mm", name="mm")
            t1s, t2s = [], []
            for mh in range(2):
                m = 2 * mp + mh
                t1 = acc.tile([P, 3], f32, tag="t1")
                t2 = acc.tile([P, 3], f32, tag="t2")
                t1s.append(t1)
                t2s.append(t2)
                if m == 0:
                    # delivery phase: contraction-outer over 7 block-chains
                    # (m0 all slots + m1 slots 0,1), consuming each et chunk
                    # pair as it lands; filler matmuls bridge the arrival
                    # gaps so the PE clock stays hot.
                    warmup(7)
                    pA = psum.tile([P, 2, JW], f32, tag="pt2", name="pA")
                    pB = psum.tile([P, 2, JW], f32, tag="pt2", name="pB")
                    pC = psum1.tile([P, JW], f32, tag="pt1", name="pC")
                    pD = psum.tile([P, 2, JW], f32, tag="pt2", name="pD")
                    for k2 in range(KC // 2):
                        st, sp = k2 == 0, k2 == KC // 2 - 1
                        for w2 in range(2):
                            mmul(pA[:, w2, :], 0, 0 + w2, k2, st, sp)
                        mmul(pC, 0, 2, k2, st, sp)
                        for w2 in range(2):
                            mmul(pB[:, w2, :], 0, 3 + w2, k2, st, sp)
                        for w2 in range(2):
                            mmul(pD[:, w2, :], 1, 0 + w2, k2, st, sp)
                        if not sp:
                            warmup(4)
                    # pC first: m1's s2 chain reuses its PSUM bank, so its
                    # exp must clear before the first post-delivery PE work
                    expmask(es, mm, pC, 0, 0, 2, 1, t1, t2, 1)
                    expmask(es, mm, pA, 0, 0, 0, 2, t1, t2, 0)
                    expmask(es, mm, pB, 0, 0, 3, 2, t1, t2, 2)
                elif m == 1:
                    # slots 0,1 were computed in the delivery phase (pD)
                    expmask(es, mm, pD, 1, 1, 0, 2, t1, t2, 0)
                    pF = psum1.tile([P, JW], f32, tag="pt1", name="pF")
                    for k2 in range(KC // 2):
                        mmul(pF, 1, 2, k2, k2 == 0, k2 == KC // 2 - 1)
                    expmask(es, mm, pF, 1, 1, 2, 1, t1, t2, 1)
                    pE = psum.tile([P, 2, JW], f32, tag="pt2", name="pE")
                    for k2 in range(KC // 2):
                        for w2 in range(2):
                            mmul(pE[:, w2, :], 1, 3 + w2, k2,
                                 k2 == 0, k2 == KC // 2 - 1)
                    expmask(es, mm, pE, 1, 1, 3, 2, t1, t2, 2)
                else:
                    for g, (sp_, w) in ((0, (0, 2)), (1, (2, 1)),
                                        (2, (3, 2))):
                        if w == 2:
                            pt = psum.tile([P, 2, JW], f32, tag="pt2",
                                           name="pt")
                            for k2 in range(KC // 2):
                                for w2 in range(2):
                                    mmul(pt[:, w2, :], m, sp_ + w2, k2,
                                         k2 == 0, k2 == KC // 2 - 1)
                        else:
                            # m3's single chain diverts to the 2-bank pool
                            # so the pos-diag chain gets the 1-bank pool
                            # (its previous user's exp clears much earlier)
                            if m == 3:
                                pt = psum.tile([P, 2, JW], f32, tag="pt2",
                                               name="pt")[:, 0, :]
                            else:
                                pt = psum1.tile([P, JW], f32, tag="pt1",
                                                name="pt")
                            for k2 in range(KC // 2):
                                mmul(pt, m, sp_, k2,
                                     k2 == 0, k2 == KC // 2 - 1)
                        expmask(es, mm, pt, m, mh, sp_, w, t1, t2, g)
                # nsout[:, m] = sum_slots(t1 - t2)
                d5 = acc.tile([P, 3], f32, tag="d5")
                nc.vector.scalar_tensor_tensor(
                    d5, t1, 1.0, t2, op0=OP.mult, op1=OP.subtract,
                    accum_out=nsout[:, m:m + 1],
                )
                if m < 3:
                    emit_pdo(m)
            # column sums for the foreign blocks (slots 0..2 = dist 1..3):
            # DoubleRow over the stacked m-chunk pair, one-hot weights land
            # dist d's sums in PSUM partition d-1 (exp) / 3+d-1 (masked).
            for d in range(1, 4):
                nc.tensor.matmul(
                    cs[0:6, :], wcs[d - 1][:, :, 0:6], es[:, :, d - 1, :],
                    start=(mp == 0 and d == 1), stop=False, perf_mode=DR,
                )
            if mp == 1:
                # pdout[:, 3] via PE: diag of E_m3 @ F_m3^T, extracted with
                # an eye mask; keeps the last pos-dot off the Vector tail
                ppos = psum1.tile([P, P], f32, tag="pt1", name="ppos")
                for k in range(KC):
                    nc.tensor.matmul(
                        ppos, et[:, k, 3 * P:4 * P], ef8[:, k, :],
                        start=(k == 0), stop=(k == KC - 1),
                    )
            for d in range(1, 4):
                nc.tensor.matmul(
                    cs[0:6, :], wcs[3 + d - 1][:, :, 0:6], mm[:, :, d - 1, :],
                    start=False, stop=(mp == 1 and d == 3), perf_mode=DR,
                )
            if mp == 1:
                scr = work.tile([P, P], bf16, tag="scr", name="scr")
                nc.vector.scalar_tensor_tensor(
                    scr, ppos, 1.0 / (SCALE * SCALE), eye,
                    op0=OP.mult, op1=OP.mult,
                    accum_out=pdout[:, 3:4],
                )
        # evict column sums (DMA cannot read PSUM)
        csev = const.tile([P, JW], f32, tag="csev")
        nc.scalar.copy(csev[0:6, :], cs[0:6, :])
        nc.sync.dma_start(out=ro_d[:, :], in_=rowout)
        nc.sync.dma_start(out=cs_d[:, :], in_=csev[0:6, :])

    nc.compile()
    return nc


def _get_program():
    if "nc" not in _CACHE:
        _CACHE["nc"] = _build_program()
    return _CACHE["nc"]


def _host_prep(layer_embeds, y_true):
    E = np.asarray(layer_embeds, dtype=np.float32)
    y = np.asarray(y_true).astype(np.int32)

    norms = np.maximum(np.linalg.norm(E, axis=1), EPS).astype(np.float32)
    Ehf = E / norms[:, None]
    Eh = Ehf.astype(BF16)
    Eh8T = np.ascontiguousarray((Ehf * SCALE).astype(FP8).T)  # [D, N]

    same = y[:, None] == y[None, :]
    nsame = same.sum(1)
    haspos = nsame > 1
    np.fill_diagonal(same, False)
    fp = np.argmax(same, axis=1)                      # first positive (j order)
    yb16 = y.astype(BF16)

    in_maps = []
    for c in range(NCORES):
        r0, r1 = c * R, (c + 1) * R
        blkcols = [np.arange(((c + b) % NCORES) * R, ((c + b) % NCORES) * R + R)
                   for b in range(NB)]
        cols = np.concatenate(blkcols)
        etc = np.ascontiguousarray(Eh8T[:, cols]).reshape(KC, P, JCOLS)
        # yt follows the slot (processing) order, et stays in block order
        ytcols = np.concatenate([blkcols[b] for b in SLOT2BLK])
        ytc = np.ascontiguousarray(
            np.broadcast_to(yb16[ytcols][None, :], (P, JCOLS)))
        enc = Eh[r0:r1].reshape(MI, P, D)
        f3 = np.ascontiguousarray(
            (Ehf[fp[r0 + 3 * P:r1]] * SCALE).astype(FP8).T).reshape(KC, P, P)
        efc = Eh[fp[r0:r1]].reshape(MI, P, D)
        in_maps.append({
            "et": etc,
            "yt": ytc,
            "yb": np.ascontiguousarray(y[r0:r1].astype(np.float32)
                                       .reshape(MI, P).T),
            "enef": np.ascontiguousarray(
                np.concatenate([enc, efc], axis=2)),
            "ef8": f3,
            "eye": np.eye(P, dtype=FP8),
        })
    meta = {"haspos": haspos, "nsame": nsame, "fp": fp}
    return in_maps, meta


def _assemble(results, meta):
    """Combine per-core partials into the scalar loss (O(N) host math)."""
    haspos = meta["haspos"]
    nsame = meta["nsame"]

    neg = np.zeros(N, dtype=np.float64)   # (T1 - T2) per row
    posd = np.zeros(N, dtype=np.float64)  # <e_i, e_fp(i)>
    for c in range(NCORES):
        r = results[c]
        rows = np.arange(c * R, (c + 1) * R)
        ro = np.asarray(r["rowout"], np.float64)
        neg[rows] += ro[:, 0:MI].T.reshape(-1)
        posd[rows] += ro[:, MI:2 * MI].T.reshape(-1)
        cso = np.asarray(r["csout"], np.float64)      # [6, JW]
        for d in range(1, 4):
            b = (c + d) % NCORES
            rows_b = np.arange(b * R, b * R + R)
            # partition d-1 holds exp colsums, 3+d-1 the masked colsums of
            # the distance-d block; JW == R so they map 1:1 onto b's rows
            neg[rows_b] += cso[d - 1, :] - cso[3 + d - 1, :]

    posS = (posd + 1.0) * 0.25
    nneg = N - nsame
    total = neg + np.where(haspos, np.exp(posS), 1.0) + (2 * N - 2 - nneg)
    posval = np.where(haspos, posS, 0.0)
    loss = float(np.mean(np.log(total) - posval))
    return np.float32(loss)


def _install_ntff_shim():
    """Provide antenv.axon_hooks (absent in this image) so trace=True works."""
    import importlib
    import types
    try:
        importlib.import_module("antenv.axon_hooks")
        return
    except ImportError:
        pass
    try:
        import antenv
        from trn_agent_boot.trn_boot import _ntff_profile_via_ctypes

        hook = _ntff_profile_via_ctypes("/opt/axon/libaxon_pjrt.so")
        mod = types.ModuleType("antenv.axon_hooks")
        mod._hook = hook
        mod.get_axon_ntff_profile_hook = lambda: mod._hook
        mod.set_axon_ntff_profile_hook = lambda h: setattr(mod, "_hook", h)
        sys.modules["antenv.axon_hooks"] = mod
        antenv.axon_hooks = mod
    except Exception as e:  # profiling is best-effort
        print(f"ntff shim failed: {e}")


def kernel(layer_embeds, y_true, _trace=False):
    import time

    if _trace:
        _install_ntff_shim()
    nc = _get_program()
    in_maps, meta = _host_prep(layer_embeds, y_true)
    last_err = None
    for attempt in range(4):
        try:
            res = run_bass_kernel_spmd(
                nc, in_maps, core_ids=list(range(NCORES)), trace=_trace,
            )
            loss = _assemble(res.results, meta)
            # lse is bounded by log(2N-2) .. log(2N + N*e^0.5) for this
            # problem shape; anything outside is transient corruption.
            if not (np.isfinite(loss) and 5.0 < float(loss) < 20.0):
                raise RuntimeError(f"implausible loss {loss}, retrying")
            if _trace:
                return loss, res
            return loss
        except Exception as e:  # transient device faults: retry
            last_err = e
            time.sleep(5 * (attempt + 1))
    raise last_err


# revision 35
# speedup vs baseline: 1.0388x; 1.0388x over previous
"""Trainium2 Bass kernel for the contrastive loss problem.

Math reformulation of the reference (no [N, 2N-1] scatter needed):
  lse_i = log( exp(pos_val_i) + sum_{j in neg} exp(S_ij) + (2N-2-num_neg_i) )
  loss  = mean_i (lse_i - pos_val_i)
with S = (cos + 1) * 0.25, cos from row-normalized embeddings.

Sharding uses the Gram matrix's symmetry: core c computes only the
[512, 512*5] strip of exp(S) pairing its rows with block-columns
{c, c+1, .., c+4} (mod 8). Columns are pre-rotated on the host so the
program is identical on every core (SPMD). Row sums cover the strip;
one-hot-weight matmuls produce per-column sums for the foreign blocks
(distance 1..3), which the host adds to those rows' totals. Distance-4
blocks are computed by both endpoint cores (row sums only).

Main matmul: fp8 e4m3 DoubleRow (K=256/op) on x16-prescaled unit rows.
Input delivery (et, 2.6MB) is HBM-bound (~10us): the first row-chunk
pair is computed contraction-outer over 7 block-chains (PSUM-capped)
so each et chunk pair is consumed as it lands, with filler matmuls
holding the PE p-state between chunk arrivals. Blocks are processed in
order [1,2,3,0,4] so the column-sum operands are ready early; exp and
mask run as paired 1024-wide ops; exp output is stored fp8 stacked
across m-chunk pairs so the column-sum matmuls are DoubleRow with
one-hot weights into a single PSUM bank (partitions 0:6).

Host: norms, fp8/bf16 casts, rotation, first-positive gather (label
metadata), final assembly of ~4096 scalars.
"""

import sys

sys.path.insert(0, "/opt/trn_rl_repo")

from contextlib import ExitStack

import ml_dtypes
import numpy as np

import concourse.bacc as bacc
import concourse.tile as tile
from concourse import mybir
from concourse.bass_utils import run_bass_kernel_spmd

N, D = 4096, 1024
NCORES = 8
R = N // NCORES            # 512 rows per core
P = 128                    # partitions
MI = R // P                # 4 row chunks per core
KC = D // P                # 8 contraction chunks
JW = 512                   # j tile width (one PSUM bank)
NB = 5                     # block-columns per core (self + 4 right neighbors)
JCOLS = NB * JW            # 2560
EPS = 1e-8
BF16 = ml_dtypes.bfloat16
FP8 = ml_dtypes.float8_e4m3
SCALE = 16.0
SLOT2BLK = [1, 2, 3, 0, 4]  # processing order: colsum blocks first

_CACHE = {}


def _build_program():
    nc = bacc.Bacc("TRN2", target_bir_lowering=False, debug=False)
    f32, bf16, fp8 = mybir.dt.float32, mybir.dt.bfloat16, mybir.dt.float8e4
    AF = mybir.ActivationFunctionType
    OP = mybir.AluOpType
    DR = mybir.MatmulPerfMode.DoubleRow

    et_d = nc.dram_tensor("et", [P, KC, JCOLS], fp8, kind="ExternalInput")
    yt_d = nc.dram_tensor("yt", [P, JCOLS], bf16, kind="ExternalInput")
    yb_d = nc.dram_tensor("yb", [P, MI], f32, kind="ExternalInput")
    enef_d = nc.dram_tensor("enef", [MI, P, 2 * D], bf16, kind="ExternalInput")
    ef8_d = nc.dram_tensor("ef8", [KC, P, P], fp8, kind="ExternalInput")
    eye_d = nc.dram_tensor("eye", [P, P], fp8, kind="ExternalInput")
    ro_d = nc.dram_tensor("rowout", [P, 2 * MI], f32, kind="ExternalOutput")
    cs_d = nc.dram_tensor("csout", [6, JW], f32, kind="ExternalOutput")

    with tile.TileContext(nc) as tc, ExitStack() as ctx:
        const = ctx.enter_context(tc.tile_pool(name="const", bufs=1))
        psum = ctx.enter_context(tc.tile_pool(name="psum", bufs=3, space="PSUM"))
        psum1 = ctx.enter_context(tc.tile_pool(name="psum1", bufs=1,
                                               space="PSUM"))
        cspsum = ctx.enter_context(tc.tile_pool(name="cspsum", bufs=1,
                                                space="PSUM"))
        esp = ctx.enter_context(tc.tile_pool(name="esp", bufs=2))
        work = ctx.enter_context(tc.tile_pool(name="work", bufs=2))
        acc = ctx.enter_context(tc.tile_pool(name="acc", bufs=2))

        et = const.tile([P, KC, JCOLS], fp8, tag="et")
        yt = const.tile([P, JCOLS], bf16, tag="yt")
        yb = const.tile([P, MI], f32, tag="yb")
        enef = const.tile([P, MI, 2, D], bf16, tag="enef")
        b025 = const.tile([P, 1], f32, tag="b025")
        nc.vector.memset(b025, 0.25)
        ones = const.tile([P, 1], bf16, tag="ones")
        nc.gpsimd.memset(ones, 1.0)
        wsrc = const.tile([P, 4 * P], bf16, tag="wsrc")
        nc.gpsimd.memset(wsrc, 1.0)
        # one-hot DoubleRow weights: wcs[d][:, :, 0:6] has a 1 in column d.
        # 16-wide padding keeps the k-pair stride a multiple of 16 (DR ISA).
        wcs = [const.tile([P, 2, 16], fp8, tag=f"wc{d}", name=f"wc{d}")
               for d in range(6)]
        for d in range(6):
            nc.gpsimd.memset(wcs[d], 0.0)
            nc.gpsimd.memset(wcs[d][:, :, d:d + 1], 1.0)
        rowout = const.tile([P, 2 * MI], f32, tag="rowout")
        nsout = rowout[:, 0:MI]
        pdout = rowout[:, MI:2 * MI]
        cs = cspsum.tile([P, JW], f32, tag="cs")

        def warmup(n):
            # keep the PE clock hot while waiting on input DMAs: tiny
            # matmuls into a partition strip the column sums never touch
            for _ in range(n):
                nc.tensor.matmul(
                    cs[96:97, 0:4 * P], ones, wsrc, start=True, stop=True,
                    tile_position=(0, 96), skip_group_check=True,
                )

        # Input DMAs on the sync HW DGE queue (HBM bandwidth is the shared
        # cap; a second queue doesn't speed aggregate delivery). et chunks
        # first, in contraction order; yt split so the first masks can
        # start right at delivery end.
        for k2 in range(KC // 2):
            nc.sync.dma_start(out=et[:, 2 * k2:2 * k2 + 2, :],
                              in_=et_d[:, 2 * k2:2 * k2 + 2, :])
        nc.sync.dma_start(out=yt[:, 0:3 * JW], in_=yt_d[:, 0:3 * JW])
        nc.sync.dma_start(out=yb, in_=yb_d[:])
        nc.sync.dma_start(out=yt[:, 3 * JW:JCOLS], in_=yt_d[:, 3 * JW:JCOLS])
        for m in range(MI):
            nc.sync.dma_start(out=enef[:, m, :, :], in_=enef_d[m])
        ef8 = const.tile([P, KC, P], fp8, tag="ef8")
        eye = const.tile([P, P], fp8, tag="eye")
        nc.sync.dma_start(out=ef8, in_=ef8_d[:, :, :])
        nc.sync.dma_start(out=eye, in_=eye_d[:, :])
        # load the Exp table while waiting on DMAs
        warm = const.tile([P, 1], f32, tag="warm")
        nc.scalar.activation(warm, b025, AF.Exp, bias=b025, scale=1.0)

        def mmul(pt, m, s, k2, start, stop):
            b = SLOT2BLK[s]
            nc.tensor.matmul(
                pt,
                et[:, 2 * k2:2 * k2 + 2, m * P:(m + 1) * P],
                et[:, 2 * k2:2 * k2 + 2, b * JW:(b + 1) * JW],
                start=start, stop=stop, perf_mode=DR,
            )

        def expmask(esv, mmv, pt, m, mh, s, w, t1, t2, slot):
            # expS = exp(cos*0.25 + 0.25); t1[:, slot] = row-sum
            nc.scalar.activation(
                esv[:, mh, s:s + w, :], pt, AF.Exp, bias=b025,
                scale=0.25 / (SCALE * SCALE),
                accum_out=t1[:, slot:slot + 1],
            )
            # t2[:, slot] = row-sum((y == y_row) * expS)
            nc.vector.scalar_tensor_tensor(
                mmv[:, mh, s:s + w, :], yt[:, s * JW:(s + w) * JW],
                yb[:, m:m + 1], esv[:, mh, s:s + w, :],
                op0=OP.is_equal, op1=OP.mult,
                accum_out=t2[:, slot:slot + 1],
            )

        def emit_pdo(mq):
            # pdout[:, mq] = row-wise <e_i, e_firstpos(i)>; m=3's dot is
            # slotted before the trailing mask so it never gates the
            # column-sum chain nor the rowout DMA
            pdo = work.tile([P, D], bf16, tag="pdo", name="pdo")
            nc.vector.scalar_tensor_tensor(
                pdo, enef[:, mq, 0, :], 1.0, enef[:, mq, 1, :],
                op0=OP.mult, op1=OP.mult,
                accum_out=pdout[:, mq:mq + 1],
            )

        for mp in range(2):
            es = esp.tile([P, 2, NB, JW], fp8, tag="es", name="es")
            mm = esp.tile([P, 2, NB, JW], fp8, tag="mm", name="mm")
            t1s, t2s = [], []
            for mh in range(2):
                m = 2 * mp + mh
                t1 = acc.tile([P, 3], f32, tag="t1")
                t2 = acc.tile([P, 3], f32, tag="t2")
                t1s.append(t1)
                t2s.append(t2)
                if m == 0:
                    # delivery phase: contraction-outer over 7 block-chains
                    # (m0 all slots + m1 slots 0,1), consuming each et chunk
                    # pair as it lands; filler matmuls bridge the arrival
                    # gaps so the PE clock stays hot.
                    warmup(7)
                    pA = psum.tile([P, 2, JW], f32, tag="pt2", name="pA")
                    pB = psum.tile([P, 2, JW], f32, tag="pt2", name="pB")
                    pC = psum1.tile([P, JW], f32, tag="pt1", name="pC")
                    pD = psum.tile([P, 2, JW], f32, tag="pt2", name="pD")
                    for k2 in range(KC // 2):
                        st, sp = k2 == 0, k2 == KC // 2 - 1
                        for w2 in range(2):
                            mmul(pA[:, w2, :], 0, 0 + w2, k2, st, sp)
                        mmul(pC, 0, 2, k2, st, sp)
                        for w2 in range(2):
                            mmul(pB[:, w2, :], 0, 3 + w2, k2, st, sp)
                        for w2 in range(2):
                            mmul(pD[:, w2, :], 1, 0 + w2, k2, st, sp)
                        if not sp:
                            warmup(4)
                    # pC first: m1's s2 chain reuses its PSUM bank, so its
                    # exp must clear before the first post-delivery PE work
                    expmask(es, mm, pC, 0, 0, 2, 1, t1, t2, 1)
                    expmask(es, mm, pA, 0, 0, 0, 2, t1, t2, 0)
                    expmask(es, mm, pB, 0, 0, 3, 2, t1, t2, 2)
                elif m == 1:
                    # slots 0,1 were computed in the delivery phase (pD)
                    expmask(es, mm, pD, 1, 1, 0, 2, t1, t2, 0)
                    pF = psum1.tile([P, JW], f32, tag="pt1", name="pF")
                    for k2 in range(KC // 2):
                        mmul(pF, 1, 2, k2, k2 == 0, k2 == KC // 2 - 1)
                    expmask(es, mm, pF, 1, 1, 2, 1, t1, t2, 1)
                    pE = psum.tile([P, 2, JW], f32, tag="pt2", name="pE")
                    for k2 in range(KC // 2):
                        for w2 in range(2):
                            mmul(pE[:, w2, :], 1, 3 + w2, k2,
                                 k2 == 0, k2 == KC // 2 - 1)
                    expmask(es, mm, pE, 1, 1, 3, 2, t1, t2, 2)
                else:
                    for g, (sp_, w) in ((0, (0, 2)), (1, (2, 1)),
                                        (2, (3, 2))):
                        if w == 2:
                            pt = psum.tile([P, 2, JW], f32, tag="pt2",
                                           name="pt")
                            for k2 in range(KC // 2):
                                for w2 in range(2):
                                    mmul(pt[:, w2, :], m, sp_ + w2, k2,
                                         k2 == 0, k2 == KC // 2 - 1)
                        else:
                            # m3's single chain diverts to the 2-bank pool
                            # so the pos-diag chain gets the 1-bank pool
                            # (its previous user's exp clears much earlier)
                            if m == 3:
                                pt = psum.tile([P, 2, JW], f32, tag="pt2",
                                               name="pt")[:, 0, :]
                            else:
                                pt = psum1.tile([P, JW], f32, tag="pt1",
                                                name="pt")
                            for k2 in range(KC // 2):
                                mmul(pt, m, sp_, k2,
                                     k2 == 0, k2 == KC // 2 - 1)
                        expmask(es, mm, pt, m, mh, sp_, w, t1, t2, g)
                # nsout[:, m] = sum_slots(t1 - t2)
                d5 = acc.tile([P, 3], f32, tag="d5")
                nc.vector.scalar_tensor_tensor(
                    d5, t1, 1.0, t2, op0=OP.mult, op1=OP.subtract,
                    accum_out=nsout[:, m:m + 1],
                )
                if m < 3:
                    emit_pdo(m)
            # column sums for the foreign blocks (slots 0..2 = dist 1..3):
            # DoubleRow over the stacked m-chunk pair, one-hot weights land
            # dist d's sums in PSUM partition d-1 (exp) / 3+d-1 (masked).
            for d in range(1, 4):
                nc.tensor.matmul(
                    cs[0:6, :], wcs[d - 1][:, :, 0:6], es[:, :, d - 1, :],
                    start=(mp == 0 and d == 1), stop=False, perf_mode=DR,
                )
            if mp == 1:
                # pdout[:, 3] via PE: diag of E_m3 @ F_m3^T, extracted with
                # an eye mask; keeps the last pos-dot off the Vector tail
                ppos = psum1.tile([P, P], f32, tag="pt1", name="ppos")
                for k in range(KC):
                    nc.tensor.matmul(
                        ppos, et[:, k, 3 * P:4 * P], ef8[:, k, :],
                        start=(k == 0), stop=(k == KC - 1),
                    )
            for d in range(1, 4):
                nc.tensor.matmul(
                    cs[0:6, :], wcs[3 + d - 1][:, :, 0:6], mm[:, :, d - 1, :],
                    start=False, stop=(mp == 1 and d == 3), perf_mode=DR,
                )
            if mp == 1:
                scr = work.tile([P, P], bf16, tag="scr", name="scr")
                nc.vector.scalar_tensor_tensor(
                    scr, ppos, 1.0 / (SCALE * SCALE), eye,
                    op0=OP.mult, op1=OP.mult,
                    accum_out=pdout[:, 3:4],
                )
        # evict column sums (DMA cannot read PSUM)
        csev = const.tile([P, JW], f32, tag="csev")
        nc.scalar.copy(csev[0:6, :], cs[0:6, :])
        nc.sync.dma_start(out=ro_d[:, :], in_=rowout)
        nc.sync.dma_start(out=cs_d[:, :], in_=csev[0:6, :])

    nc.compile()
    return nc


def _get_program():
    if "nc" not in _CACHE:
        _CACHE["nc"] = _build_program()
    return _CACHE["nc"]


def _host_prep(layer_embeds, y_true):
    E = np.asarray(layer_embeds, dtype=np.float32)
    y = np.asarray(y_true).astype(np.int32)

    norms = np.maximum(np.linalg.norm(E, axis=1), EPS).astype(np.float32)
    Ehf = E / norms[:, None]
    Eh = Ehf.astype(BF16)
    Eh8T = np.ascontiguousarray((Ehf * SCALE).astype(FP8).T)  # [D, N]

    same = y[:, None] == y[None, :]
    nsame = same.sum(1)
    haspos = nsame > 1
    np.fill_diagonal(same, False)
    fp = np.argmax(same, axis=1)                      # first positive (j order)
    yb16 = y.astype(BF16)

    in_maps = []
    for c in range(NCORES):
        r0, r1 = c * R, (c + 1) * R
        blkcols = [np.arange(((c + b) % NCORES) * R, ((c + b) % NCORES) * R + R)
                   for b in range(NB)]
        cols = np.concatenate(blkcols)
        etc = np.ascontiguousarray(
            Eh8T[:, cols].reshape(KC, P, JCOLS).transpose(1, 0, 2))
        # yt follows the slot (processing) order, et stays in block order
        ytcols = np.concatenate([blkcols[b] for b in SLOT2BLK])
        ytc = np.ascontiguousarray(
            np.broadcast_to(yb16[ytcols][None, :], (P, JCOLS)))
        enc = Eh[r0:r1].reshape(MI, P, D)
        f3 = np.ascontiguousarray(
            (Ehf[fp[r0 + 3 * P:r1]] * SCALE).astype(FP8).T).reshape(KC, P, P)
        efc = Eh[fp[r0:r1]].reshape(MI, P, D)
        in_maps.append({
            "et": etc,
            "yt": ytc,
            "yb": np.ascontiguousarray(y[r0:r1].astype(np.float32)
                                       .reshape(MI, P).T),
            "enef": np.ascontiguousarray(
                np.concatenate([enc, efc], axis=2)),
            "ef8": f3,
            "eye": np.eye(P, dtype=FP8),
        })
    meta = {"haspos": haspos, "nsame": nsame, "fp": fp}
    return in_maps, meta


def _assemble(results, meta):
    """Combine per-core partials into the scalar loss (O(N) host math)."""
    haspos = meta["haspos"]
    nsame = meta["nsame"]

    neg = np.zeros(N, dtype=np.float64)   # (T1 - T2) per row
    posd = np.zeros(N, dtype=np.float64)  # <e_i, e_fp(i)>
    for c in range(NCORES):
        r = results[c]
        rows = np.arange(c * R, (c + 1) * R)
        ro = np.asarray(r["rowout"], np.float64)
        neg[rows] += ro[:, 0:MI].T.reshape(-1)
        posd[rows] += ro[:, MI:2 * MI].T.reshape(-1)
        cso = np.asarray(r["csout"], np.float64)      # [6, JW]
        for d in range(1, 4):
            b = (c + d) % NCORES
            rows_b = np.arange(b * R, b * R + R)
            # partition d-1 holds exp colsums, 3+d-1 the masked colsums of
            # the distance-d block; JW == R so they map 1:1 onto b's rows
            neg[rows_b] += cso[d - 1, :] - cso[3 + d - 1, :]

    posS = (posd + 1.0) * 0.25
    nneg = N - nsame
    total = neg + np.where(haspos, np.exp(posS), 1.0) + (2 * N - 2 - nneg)
    posval = np.where(haspos, posS, 0.0)
    loss = float(np.mean(np.log(total) - posval))
    return np.float32(loss)


def _install_ntff_shim():
    """Provide antenv.axon_hooks (absent in this image) so trace=True works."""
    import importlib
    import types
    try:
        importlib.import_module("antenv.axon_hooks")
        return
    except ImportError:
        pass
    try:
        import antenv
        from trn_agent_boot.trn_boot import _ntff_profile_via_ctypes

        hook = _ntff_profile_via_ctypes("/opt/axon/libaxon_pjrt.so")
        mod = types.ModuleType("antenv.axon_hooks")
        mod._hook = hook
        mod.get_axon_ntff_profile_hook = lambda: mod._hook
        mod.set_axon_ntff_profile_hook = lambda h: setattr(mod, "_hook", h)
        sys.modules["antenv.axon_hooks"] = mod
        antenv.axon_hooks = mod
    except Exception as e:  # profiling is best-effort
        print(f"ntff shim failed: {e}")


def kernel(layer_embeds, y_true, _trace=False):
    import time

    if _trace:
        _install_ntff_shim()
    nc = _get_program()
    in_maps, meta = _host_prep(layer_embeds, y_true)
    last_err = None
    for attempt in range(4):
        try:
            res = run_bass_kernel_spmd(
                nc, in_maps, core_ids=list(range(NCORES)), trace=_trace,
            )
            loss = _assemble(res.results, meta)
            # lse is bounded by log(2N-2) .. log(2N + N*e^0.5) for this
            # problem shape; anything outside is transient corruption.
            if not (np.isfinite(loss) and 5.0 < float(loss) < 20.0):
                raise RuntimeError(f"implausible loss {loss}, retrying")
            if _trace:
                return loss, res
            return loss
        except Exception as e:  # transient device faults: retry
            last_err = e
            time.sleep(5 * (attempt + 1))
    raise last_err
